# revision 1
# baseline (speedup 1.0000x reference)
"""Trainium2 Bass kernel for nn_AmplifierAttention (sparse sigmoid-threshold attention).

  t    = text @ W^T + b          [S, E]
  attn = t @ L^T                 [S, Lb]
  a    = sigmoid(attn); a[a < 0.4] = 0
  out  = softmax(a, axis=-1) @ L [S, E]

Strategy: data-parallel over batch B=8 -> one batch element per NeuronCore,
weights replicated, zero collectives.  Per core, everything is computed
transposed (contraction dims on partitions) so no on-chip transposes are
needed; the host pre-transposes text/W/L instead.

Softmax trick: softmax(a) @ L == (exp(a - c) @ L) / rowsum(exp(a - c)) for any
constant c.  With h = tanh(attn/2) (same ACT table set as exp), sigmoid =
(h+1)/2 and the thresholded exponent is exp(0.5*u - 0.5) where
u = (h+1)*[h >= -0.2]  (sigmoid(x) < 0.4  <=>  tanh(x/2) < -0.2).
The rowsum comes free from a ones-column appended to L in the last matmul.
All matmul inputs bf16 (fp32 PSUM accumulation): measured rel err ~7e-3.
"""

import os
import numpy as np
import ml_dtypes

P = 128
BF = ml_dtypes.bfloat16

_PROGRAM_CACHE = {}


def build_program(S=2048, DT=1024, E=768, L=4096, SC=512):
    """Build + compile the per-core Bass program (same SPMD program on all cores)."""
    from contextlib import ExitStack
    import concourse.bass as bass  # noqa: F401
    import concourse.mybir as mybir
    import concourse.tile as tile
    from concourse.tile import add_dep_helper
    from concourse import bacc

    def _raw(i):
        return getattr(i, "ins", i)

    dt = mybir.dt
    AF = mybir.ActivationFunctionType
    OP = mybir.AluOpType

    ND = DT // P        # d-tiles
    NE = E // P         # e-tiles
    NL = L // P         # l-tiles
    NPAIR = NL // 2     # l-pairs (two l-tiles share one 2-bank PSUM tile)
    NCH = S // SC       # s-chunks
    NSS = SC // P       # s-subtiles per chunk
    EH = E // 2         # half of the output feature dim
    EP = E + 2          # padded label row: E features + ones col + zero pad
    SSLOT = 2 * max(SC, 512)   # psum slot width (fp32), 2 banks
    HOFF = SSLOT // 2          # bank-aligned offset of the low-half accumulator

    nc = bacc.Bacc("TRN2", target_bir_lowering=False, debug=False)

    NCH_ = S // SC
    # tt/wt arrive host-packed in SBUF tile order (one contiguous run per
    # partition per transfer) — the startup stream then runs near peak HBM
    # bandwidth instead of the ~200GB/s that 1KB strided runs achieve
    tt = nc.dram_tensor("tt", [P, NCH_, DT // P, SC], dt.bfloat16,
                        kind="ExternalInput").ap()
    wt = nc.dram_tensor("wt", [P, E // P, DT // P, P], dt.bfloat16,
                        kind="ExternalInput").ap()
    lt = nc.dram_tensor("lt", [E, L], dt.bfloat16, kind="ExternalInput").ap()
    laug = nc.dram_tensor("laug", [L, EP], dt.bfloat16, kind="ExternalInput").ap()
    bb = nc.dram_tensor("bb", [E], dt.float32, kind="ExternalInput").ap()
    out = nc.dram_tensor("out", [S, E], dt.float32, kind="ExternalOutput").ap()

    with tile.TileContext(nc) as tc, ExitStack() as ctx:
        const_pool = ctx.enter_context(tc.tile_pool(name="const", bufs=1))
        tt_pool = ctx.enter_context(tc.tile_pool(name="ttp", bufs=1))
        t_pool = ctx.enter_context(tc.tile_pool(name="tp", bufs=1))
        w_pool = ctx.enter_context(tc.tile_pool(name="wp", bufs=1))
        ew_pool = ctx.enter_context(tc.tile_pool(name="ewp", bufs=2))
        o_pool = ctx.enter_context(tc.tile_pool(name="op", bufs=1))
        r_pool = ctx.enter_context(tc.tile_pool(name="rp", bufs=2))
        # one shared PSUM pool: 4 slots x 2 banks = all 8 banks.  The three
        # phases are PE-serial, so sharing gives attention 4-deep buffering
        # and step 3 all four accumulator pairs in a single pass.
        pp_pool = ctx.enter_context(tc.tile_pool(name="pp", bufs=4, space="PSUM"))

        # --- resident weights.  Everything rides the sync HWDGE ring, which
        # is FIFO per issuing engine — so emission order IS bandwidth
        # priority: step-1 critical tensors (wt, chunk-0 text) first, then
        # the label tensors (first needed ~20us / ~60us in).
        # step 1 runs d-outer over e-PAIRS; the DMA order matches the
        # consumption order exactly: first weight pair, then the text in
        # quarters, then the remaining weight pairs — the first matmul can
        # start after just 0.6MB and never stalls mid-group
        wt_sb = const_pool.tile([P, NE, ND, P], dt.bfloat16, tag="wt")
        EG = min(2, NE)
        nc.sync.dma_start(wt_sb[:, 0:EG], wt[:, 0:EG])
        tt0_sb = tt_pool.tile([P, ND, SC], dt.bfloat16, tag="tt")
        dstep = max(1, ND // 4)
        for d0 in range(0, ND, dstep):
            d1 = min(d0 + dstep, ND)
            nc.sync.dma_start(tt0_sb[:, d0:d1, :], tt[:, 0, d0:d1, :])
        for g0 in range(EG, NE, EG):
            g1 = min(g0 + EG, NE)
            nc.sync.dma_start(wt_sb[:, g0:g1], wt[:, g0:g1])
        b_sb = const_pool.tile([P, NE], dt.float32, tag="b")
        nc.sync.dma_start(b_sb[:], bb.rearrange("(a p) -> p a", p=P))
        nbias = const_pool.tile([P, 1], dt.float32, tag="nb")
        nc.vector.memset(nbias[:], -0.5)

        tt_sbs = {0: tt0_sb}

        # lt streams in ascending-size l-pieces (all e-tiles of a piece
        # together) so the first attention pairs wait only for the first
        # ~0.75MB piece instead of the full 6MB
        lt_sb = const_pool.tile([P, NE, L], dt.bfloat16, tag="lt")
        lt_r = lt.rearrange("(a p) l -> p a l", p=P)
        lt_cuts = sorted({0, min(L // 8, L), min(L // 4, L),
                          min(L // 2, L), L})
        for lo, hi in zip(lt_cuts, lt_cuts[1:]):
            for e in range(NE):
                nc.sync.dma_start(lt_sb[:, e, lo:hi], lt_r[:, e, lo:hi])
        la_sb = const_pool.tile([P, NL, EP], dt.bfloat16, tag="la")
        la_r = laug.rearrange("(a p) e -> p a e", p=P)
        for li in range(NL):
            nc.sync.dma_start(la_sb[:, li, :], la_r[:, li, :])

        def do_step1(cc):
            # step 1: t^T[e, s] = sum_d W^T[d,e] * text^T[d,s]  (+ bias).
            # d-outer over groups of <=4 e-tiles: each arriving text d-tile
            # feeds several matmuls, so chunk-0 is not paced by the text DMA
            t_sb = t_pool.tile([P, NE, SC], dt.bfloat16, tag="t",
                               name=f"t_{cc}")
            prev_anchor = None
            for eg0 in range(0, NE, 2):
                eg = range(eg0, min(eg0 + 2, NE))
                pss = {e: pp_pool.tile([P, SSLOT], dt.float32, tag="pp",
                                       name=f"ps_{cc}_{e}") for e in eg}
                for d in range(ND):
                    for e in eg:
                        m = nc.tensor.matmul(
                            pss[e][:, :SC],
                            lhsT=wt_sb[:, e, d, :],
                            rhs=tt_sbs[cc][:, d, :],
                            start=(d == 0), stop=(d == ND - 1),
                        )
                        if prev_anchor is not None:
                            # stop the scheduler hoisting this group's
                            # slot-waiting matmuls ahead of the work whose
                            # evacuation releases the slots
                            add_dep_helper(_raw(m), _raw(prev_anchor),
                                           sync=False,
                                           reason="step1 group order")
                            prev_anchor = None
                for e in eg:
                    last_evac = nc.scalar.activation(
                        t_sb[:, e, :], pss[e][:, :SC],
                        AF.Identity, bias=b_sb[:, e:e + 1])
                prev_anchor = last_evac
            return t_sb, last_evac

        for c in range(NCH):
            s0 = c * SC
            t_sb, anchor = do_step1(c)
            if c + 1 < NCH:
                # prefetch next chunk's text now so its DMA sits ahead of
                # this chunk's output stores on the sync FIFO; the ordering
                # edge stops the scheduler hoisting it ahead of the loads
                # whose consumers release its slot
                tt_sbs[c + 1] = tt_pool.tile([P, ND, SC], dt.bfloat16,
                                             tag="tt", name=f"tt{c + 1}")
                d = nc.sync.dma_start(tt_sbs[c + 1][:],
                                      tt[:, c + 1])
                add_dep_helper(_raw(d), _raw(anchor), sync=False,
                               reason="tt prefetch after this chunk's step1")

            # ---- step 2: attn^T[l, s] per l-pair + elementwise -> w
            w_sb = w_pool.tile([P, NPAIR, 2 * SC], dt.bfloat16, tag="w")
            for pr in range(NPAIR):
                pa_full = pp_pool.tile([P, SSLOT], dt.float32, tag="pp")
                pa = pa_full[:, :2 * SC]
                for sub in range(2):
                    li = 2 * pr + sub
                    for e in range(NE):
                        nc.tensor.matmul(
                            pa[:, sub * SC:(sub + 1) * SC],
                            lhsT=lt_sb[:, e, li * P:(li + 1) * P],
                            rhs=t_sb[:, e, :],
                            start=(e == 0), stop=(e == NE - 1),
                        )
                h = ew_pool.tile([P, 2 * SC], dt.bfloat16, tag="h")
                nc.scalar.activation(h[:], pa[:], AF.Tanh, scale=0.5)
                hp1 = ew_pool.tile([P, 2 * SC], dt.bfloat16, tag="hp1")
                nc.vector.tensor_scalar(hp1[:], h[:], 1.0, None, OP.add)
                msk = ew_pool.tile([P, 2 * SC], dt.bfloat16, tag="m")
                nc.vector.tensor_scalar(msk[:], h[:], -0.2, None, OP.is_ge)
                u = ew_pool.tile([P, 2 * SC], dt.bfloat16, tag="u")
                nc.vector.tensor_tensor(u[:], hp1[:], msk[:], OP.mult)
                nc.scalar.activation(w_sb[:, pr, :], u[:], AF.Exp,
                                     bias=nbias[:], scale=0.5)

            # ---- step 3: out[s, :] = (w @ [L | 1]) / rowsum.  One psum slot
            # per s-subtile holds both e-half accumulators (bank-aligned
            # halves), so all NSS subtiles run in a single pass over l and
            # the two matmuls sharing one stationary w-tile are adjacent.
            # The upper half carries the ones column -> rowsum; its
            # evacuation goes to ScalarE (Copy with per-partition scale)
            # while VectorE handles the lower half, so they overlap.
            out_sb = o_pool.tile([P, NSS, E], dt.float32, tag="osb")
            rinv = r_pool.tile([P, NSS], dt.float32, tag="rinv")
            for ss in range(NSS):
                slot = pp_pool.tile([P, SSLOT], dt.float32, tag="pp",
                                    name=f"po_{c}_{ss}")
                for pr in range(NPAIR):
                    for sub in range(2):
                        li = 2 * pr + sub
                        first = (li == 0)
                        last = (li == NL - 1)
                        lhsT = w_sb[:, pr, sub * SC + ss * P:
                                    sub * SC + (ss + 1) * P]
                        nc.tensor.matmul(
                            slot[:, :EH + 1], lhsT=lhsT,
                            rhs=la_sb[:, li, EH:E + 1],
                            start=first, stop=last,
                        )
                        nc.tensor.matmul(
                            slot[:, HOFF:HOFF + EH], lhsT=lhsT,
                            rhs=la_sb[:, li, 0:EH],
                            start=first, stop=last,
                        )
                # evacuate this subtile while the next one accumulates:
                # ScalarE takes the upper half, VectorE the lower half
                nc.vector.reciprocal(rinv[:, ss:ss + 1],
                                     slot[:, EH:EH + 1])
                nc.scalar.activation(out_sb[:, ss, EH:E],
                                     slot[:, :EH], AF.Copy,
                                     scale=rinv[:, ss:ss + 1])
                nc.sync.dma_start(out[s0 + ss * P:s0 + (ss + 1) * P, EH:E],
                                  out_sb[:, ss, EH:E])
                nc.vector.tensor_scalar(out_sb[:, ss, 0:EH],
                                        slot[:, HOFF:HOFF + EH],
                                        rinv[:, ss:ss + 1], None, OP.mult)
                nc.sync.dma_start(out[s0 + ss * P:s0 + (ss + 1) * P, 0:EH],
                                  out_sb[:, ss, 0:EH])

    nc.compile()
    return nc


def _get_program(key):
    if key not in _PROGRAM_CACHE:
        _PROGRAM_CACHE[key] = build_program(*key)
    return _PROGRAM_CACHE[key]


def prep_inputs(text_vec, labels_vec, W_proj, b_proj):
    """Host-side shard + layout prep: transpose/cast to the DRAM layouts the
    kernel expects.  Returns in_maps for run_bass_kernel_spmd."""
    B, S, DT = text_vec.shape
    L, E = labels_vec.shape
    # W packed e-major into SBUF tile order: [128, E/128, DT/128, 128]
    wt = np.ascontiguousarray(
        W_proj.reshape(E // 128, 128, DT // 128, 128)
        .transpose(3, 0, 2, 1)).astype(BF)
    lt = np.ascontiguousarray(labels_vec.T).astype(BF)            # [E, L]
    laug = np.zeros((L, E + 2), dtype=BF)
    laug[:, :E] = labels_vec.astype(BF)
    laug[:, E] = 1.0
    b32 = np.ascontiguousarray(b_proj).astype(np.float32)
    SC = 512
    in_maps = []
    for b in range(B):
        # text^T packed chunk-major: [128, S/SC, DT/128, SC]
        ttb = np.ascontiguousarray(
            text_vec[b].T.reshape(DT // 128, 128, S // SC, SC)
            .transpose(1, 2, 0, 3)).astype(BF)
        in_maps.append({"tt": ttb, "wt": wt, "lt": lt, "laug": laug, "bb": b32})
    return in_maps


def kernel(text_vec, labels_vec, W_proj, b_proj):
    from concourse.bass_utils import run_bass_kernel_spmd

    text_vec = np.asarray(text_vec)
    labels_vec = np.asarray(labels_vec)
    W_proj = np.asarray(W_proj)
    b_proj = np.asarray(b_proj)

    B, S, DT = text_vec.shape
    L, E = labels_vec.shape
    nc = _get_program((S, DT, E, L, 512))
    in_maps = prep_inputs(text_vec, labels_vec, W_proj, b_proj)

    trace = bool(int(os.environ.get("AMP_TRACE", "0")))
    res = run_bass_kernel_spmd(nc, in_maps, core_ids=list(range(B)), trace=trace)
    if trace and res.exec_time_ns is not None:
        print(f"HW exec time: {res.exec_time_ns} ns")
        if res.instructions_and_trace is not None:
            print(f"trace: {res.instructions_and_trace[1]}")
    out = np.stack([res.results[b]["out"] for b in range(B)], axis=0)
    return out.astype(np.float32)

